# revision 11
# baseline (speedup 1.0000x reference)
"""EMA (exponential moving average) Trainium2 kernel.

Problem: y_t = w * x_t + (1-w) * y_{t-1} over the last (time) axis of
mag_spec [B=32, C=256, T=4096], initial state [B, C, 1], scalar weight w.

Strategy: data-parallel over the batch dim across 8 NeuronCores. Each core
gets a [4, 256, 4096] slab = 1024 independent rows. Rows go on SBUF
partitions (8 tiles of [128, 4096]); the time recurrence runs along the
free dimension with the DVE hardware scan instruction
(tensor_tensor_scan: state = data0*state + data1), with data1 = w*x
computed on DVE and data0 a constant (1-w) tile. Memory-bound:
~33.5 MB of HBM traffic per core.

Raw Bass (no Tile): the TPB instruction encodings carry at most one
sync-wait and one sem-update each, and this toolchain's walrus refuses
instructions where Tile attached two waits. With explicit semaphores all
waits are standalone wait_ge instructions, so the limit never binds.

Pipeline per core (8 row-tiles): x loads on the SP HWDGE ring, y stores
on the ACT HWDGE ring, all compute on DVE (memset const, w*x scale,
hardware scan), triple-buffered xt/yt.
"""

import numpy as np

B, C, T = 32, 256, 4096
M = 8          # cores
P = 128        # SBUF partitions
R = (B // M) * C   # rows per core = 1024
NT = R // P        # row tiles per core = 8

_CACHE: dict = {}
LAST_RESULT = None  # BassKernelResults of the most recent run (for test.py)


def _build(w: float):
    from contextlib import ExitStack

    import concourse.bass as bass
    from concourse import mybir

    a = 1.0 - w
    f32 = mybir.dt.float32

    nc = bass.Bass()
    x_in = nc.dram_tensor("x", [R, T], f32, kind="ExternalInput")
    # init, host-pretransposed: s_in[p, i] = initial_state row 128*i + p
    s_in = nc.dram_tensor("init", [P, NT], f32, kind="ExternalInput")
    y_out = nc.dram_tensor("y", [R, T], f32, kind="ExternalOutput")

    with ExitStack() as ctx:
        ec = ctx.enter_context
        c_a = ec(nc.sbuf_tensor([P, T], f32))        # (1-w) broadcast tile
        sall = ec(nc.sbuf_tensor([P, NT], f32))      # init, DMA landing
        sall2 = ec(nc.sbuf_tensor([P, NT], f32))     # init, DVE-homed copy
        xts = [ec(nc.sbuf_tensor(f"xt{k}", [P, T], f32)) for k in range(3)]
        wt = ec(nc.sbuf_tensor([P, T], f32))         # w*x staging
        yts = [ec(nc.sbuf_tensor(f"yt{k}", [P, T], f32)) for k in range(3)]
        # One sem per DMA buffer slot: at most one in-flight incrementer per
        # sem, so completion-order nondeterminism across concurrent DMAs
        # can't satisfy a wait with the wrong transfer.
        cst_sem = ec(nc.semaphore())  # c_a memset done
        cp_sem = ec(nc.semaphore())   # sall2 copy done
        s_sem = ec(nc.semaphore())    # init load done
        in_sems = [ec(nc.semaphore(f"in_sem{k}")) for k in range(3)]
        mul_sem = ec(nc.semaphore())  # DVE consumed x tile i
        scan_sem = ec(nc.semaphore()) # scan i done (y tile ready)
        out_sems = [ec(nc.semaphore(f"out_sem{k}")) for k in range(3)]
        block = ec(nc.Block())

        @block.sync
        def _(sync):
            # x loads on the SP HWDGE ring
            sync.dma_start(sall[:], s_in[:]).then_inc(s_sem, 16)
            for j in range(NT):
                if j >= 3:
                    sync.wait_ge(mul_sem, j - 2)  # xt slot j%3 free
                sync.dma_start(
                    xts[j % 3][:], x_in[bass.ts(j, P), :]
                ).then_inc(in_sems[j % 3], 16)

        @block.scalar
        def _(scalar):
            # y stores on the ACT HWDGE ring
            for i in range(NT):
                scalar.wait_ge(scan_sem, i + 1)
                scalar.dma_start(
                    y_out[bass.ts(i, P), :], yts[i % 3][:]
                ).then_inc(out_sems[i % 3], 16)

        @block.vector
        def _(vector):
            # Engine pipelines are deep: even same-engine RAW/WAR hazards
            # need sem edges (the race detector enforces this).
            vector.memset(c_a[:], a).then_inc(cst_sem, 1)
            vector.wait_ge(s_sem, 16)
            vector.tensor_copy(sall2[:], sall[:]).then_inc(cp_sem, 1)
            vector.wait_ge(cst_sem, 1)
            vector.wait_ge(cp_sem, 1)
            for i in range(NT):
                vector.wait_ge(in_sems[i % 3], 16 * (i // 3 + 1))
                if i >= 1:
                    vector.wait_ge(scan_sem, i)  # WAR: wt still read by scan i-1
                vector.tensor_scalar_mul(wt[:], xts[i % 3][:], w).then_inc(
                    mul_sem, 1
                )
                if i >= 3:
                    vector.wait_ge(out_sems[i % 3], 16 * (i // 3))
                vector.wait_ge(mul_sem, i + 1)  # RAW: wt
                # y[:, t] = a * y[:, t-1] + (w*x)[:, t], seeded per-partition
                vector.tensor_tensor_scan(
                    yts[i % 3][:], c_a[:], wt[:], sall2[:, i : i + 1],
                    op0=mybir.AluOpType.mult, op1=mybir.AluOpType.add,
                ).then_inc(scan_sem, 1)
    return nc


def _run(in_maps, w: float, trace: bool = False):
    global LAST_RESULT
    from concourse.bass_utils import run_bass_kernel_spmd

    if w not in _CACHE:
        _CACHE[w] = _build(w)
    LAST_RESULT = run_bass_kernel_spmd(
        _CACHE[w], in_maps, list(range(M)), trace=trace
    )
    return LAST_RESULT.results


def kernel(mag_spec, initial_state, weights, _trace: bool = False) -> np.ndarray:
    w = float(np.clip(np.asarray(weights, dtype=np.float32).reshape(-1)[0], 0.0, 1.0))
    x = np.ascontiguousarray(np.asarray(mag_spec, dtype=np.float32)).reshape(B * C, T)
    s = np.asarray(initial_state, dtype=np.float32).reshape(B * C)
    in_maps = [
        {
            "x": np.ascontiguousarray(x[i * R : (i + 1) * R]),
            "init": np.ascontiguousarray(s[i * R : (i + 1) * R].reshape(NT, P).T),
        }
        for i in range(M)
    ]
    res = _run(in_maps, w, trace=_trace)
    y = np.concatenate([res[i]["y"] for i in range(M)], axis=0)
    return y.reshape(B, C, T)


# revision 13
# speedup vs baseline: 1.0008x; 1.0008x over previous
"""EMA (exponential moving average) Trainium2 kernel.

Problem: y_t = w * x_t + (1-w) * y_{t-1} over the last (time) axis of
mag_spec [B=32, C=256, T=4096], initial state [B, C, 1], scalar weight w.

Strategy: data-parallel over the batch dim across 8 NeuronCores. Each core
gets a [4, 256, 4096] slab = 1024 independent rows. Rows go on SBUF
partitions (8 tiles of [128, 4096]); the time recurrence runs along the
free dimension with the DVE hardware scan instruction
(tensor_tensor_scan: state = data0*state + data1), with data1 = w*x
computed on DVE and data0 a constant (1-w) tile. Memory-bound:
~33.5 MB of HBM traffic per core.

Raw Bass (no Tile): the TPB instruction encodings carry at most one
sync-wait and one sem-update each, and this toolchain's walrus refuses
instructions where Tile attached two waits. With explicit semaphores all
waits are standalone wait_ge instructions, so the limit never binds.

Pipeline per core (8 row-tiles): x loads on the SP HWDGE ring, y stores
on the ACT HWDGE ring, all compute on DVE (memset const, w*x scale,
hardware scan), triple-buffered xt/yt.
"""

import numpy as np

B, C, T = 32, 256, 4096
M = 8          # cores
P = 128        # SBUF partitions
R = (B // M) * C   # rows per core = 1024
NT = R // P        # row tiles per core = 8

_CACHE: dict = {}
LAST_RESULT = None  # BassKernelResults of the most recent run (for test.py)


def _build(w: float):
    from contextlib import ExitStack

    import concourse.bass as bass
    from concourse import mybir

    a = 1.0 - w
    f32 = mybir.dt.float32

    nc = bass.Bass()
    x_in = nc.dram_tensor("x", [R, T], f32, kind="ExternalInput")
    # init, host-pretransposed: s_in[p, i] = initial_state row 128*i + p
    s_in = nc.dram_tensor("init", [P, NT], f32, kind="ExternalInput")
    y_out = nc.dram_tensor("y", [R, T], f32, kind="ExternalOutput")

    with ExitStack() as ctx:
        ec = ctx.enter_context
        c_a = ec(nc.sbuf_tensor([P, T], f32))        # (1-w) broadcast tile
        sall = ec(nc.sbuf_tensor([P, NT], f32))      # init/w, DMA landing
        sall2 = ec(nc.sbuf_tensor([P, NT], f32))     # init/w, DVE-homed copy
        xts = [ec(nc.sbuf_tensor(f"xt{k}", [P, T], f32)) for k in range(3)]
        zts = [ec(nc.sbuf_tensor(f"zt{k}", [P, T], f32)) for k in range(2)]
        yts = [ec(nc.sbuf_tensor(f"yt{k}", [P, T], f32)) for k in range(3)]
        # One sem per DMA buffer slot: at most one in-flight incrementer per
        # sem, so completion-order nondeterminism across concurrent DMAs
        # can't satisfy a wait with the wrong transfer.
        cst_sem = ec(nc.semaphore())  # c_a memset done
        cp_sem = ec(nc.semaphore())   # sall2 copy done
        s_sem = ec(nc.semaphore())    # init load done
        in_sems = [ec(nc.semaphore(f"in_sem{k}")) for k in range(3)]
        scan_sem = ec(nc.semaphore()) # scan i done (z tile ready)
        act_sem = ec(nc.semaphore())  # ACT scale i done (y tile ready)
        out_sems = [ec(nc.semaphore(f"out_sem{k}")) for k in range(3)]
        block = ec(nc.Block())

        @block.sync
        def _(sync):
            # x loads on the SP HWDGE ring
            sync.dma_start(sall[:], s_in[:]).then_inc(s_sem, 16)
            for j in range(NT):
                if j >= 3:
                    sync.wait_ge(scan_sem, j - 2)  # xt slot j%3 free
                sync.dma_start(
                    xts[j % 3][:], x_in[bass.ts(j, P), :]
                ).then_inc(in_sems[j % 3], 16)

        @block.vector
        def _(vector):
            # Engine pipelines are deep: even same-engine RAW/WAR hazards
            # need sem edges (the race detector enforces this).
            vector.memset(c_a[:], a).then_inc(cst_sem, 1)
            vector.wait_ge(s_sem, 16)
            vector.tensor_copy(sall2[:], sall[:]).then_inc(cp_sem, 1)
            vector.wait_ge(cst_sem, 1)
            vector.wait_ge(cp_sem, 1)
            for i in range(NT):
                vector.wait_ge(in_sems[i % 3], 16 * (i // 3 + 1))
                if i >= 2:
                    vector.wait_ge(act_sem, i - 1)  # zt slot i%2 free
                # z[:, t] = a * z[:, t-1] + x[:, t], seeded with init/w
                vector.tensor_tensor_scan(
                    zts[i % 2][:], c_a[:], xts[i % 3][:], sall2[:, i : i + 1],
                    op0=mybir.AluOpType.mult, op1=mybir.AluOpType.add,
                ).then_inc(scan_sem, 1)

        @block.scalar
        def _(scalar):
            # y = w*z on ScalarE, then store on the ACT HWDGE ring
            for i in range(NT):
                scalar.wait_ge(scan_sem, i + 1)
                if i >= 3:
                    scalar.wait_ge(out_sems[i % 3], 16 * (i // 3))  # yt free
                scalar.mul(yts[i % 3][:], zts[i % 2][:], w).then_inc(act_sem, 1)
                scalar.wait_ge(act_sem, i + 1)
                scalar.dma_start(
                    y_out[bass.ts(i, P), :], yts[i % 3][:]
                ).then_inc(out_sems[i % 3], 16)
    return nc


def _run(in_maps, w: float, trace: bool = False):
    global LAST_RESULT
    from concourse.bass_utils import run_bass_kernel_spmd

    if w not in _CACHE:
        _CACHE[w] = _build(w)
    LAST_RESULT = run_bass_kernel_spmd(
        _CACHE[w], in_maps, list(range(M)), trace=trace
    )
    return LAST_RESULT.results


def kernel(mag_spec, initial_state, weights, _trace: bool = False) -> np.ndarray:
    w = float(np.clip(np.asarray(weights, dtype=np.float32).reshape(-1)[0], 0.0, 1.0))
    x = np.ascontiguousarray(np.asarray(mag_spec, dtype=np.float32)).reshape(B * C, T)
    s = np.asarray(initial_state, dtype=np.float32).reshape(B * C)
    if w == 0.0:
        # y_t = y_{t-1} = init for all t; the z = y/w formulation divides by w
        return np.broadcast_to(
            s.reshape(B, C, 1), (B, C, T)
        ).astype(np.float32).copy()
    # device scans z_t = x_t + (1-w) z_{t-1} seeded with init/w; y = w*z
    sw = (s / np.float32(w)).astype(np.float32)
    in_maps = [
        {
            "x": np.ascontiguousarray(x[i * R : (i + 1) * R]),
            "init": np.ascontiguousarray(sw[i * R : (i + 1) * R].reshape(NT, P).T),
        }
        for i in range(M)
    ]
    res = _run(in_maps, w, trace=_trace)
    y = np.concatenate([res[i]["y"] for i in range(M)], axis=0)
    return y.reshape(B, C, T)


# revision 14
# speedup vs baseline: 1.1901x; 1.1892x over previous
"""EMA (exponential moving average) Trainium2 kernel.

Problem: y_t = w * x_t + (1-w) * y_{t-1} over the last (time) axis of
mag_spec [B=32, C=256, T=4096], initial state [B, C, 1], scalar weight w.

Strategy: data-parallel over the batch dim across 8 NeuronCores. Each core
gets a [4, 256, 4096] slab = 1024 independent rows. Rows go on SBUF
partitions (8 tiles of [128, 4096]); the time recurrence runs along the
free dimension with the DVE hardware scan instruction
(tensor_tensor_scan: state = data0*state + data1), with data1 = w*x
computed on DVE and data0 a constant (1-w) tile. Memory-bound:
~33.5 MB of HBM traffic per core.

Raw Bass (no Tile): the TPB instruction encodings carry at most one
sync-wait and one sem-update each, and this toolchain's walrus refuses
instructions where Tile attached two waits. With explicit semaphores all
waits are standalone wait_ge instructions, so the limit never binds.

Pipeline per core (8 row-tiles): x loads on the SP HWDGE ring, y stores
on the ACT HWDGE ring, all compute on DVE (memset const, w*x scale,
hardware scan), triple-buffered xt/yt.
"""

import numpy as np

B, C, T = 32, 256, 4096
M = 8          # cores
P = 128        # SBUF partitions
R = (B // M) * C   # rows per core = 1024
XBUF = 5           # x-tile double buffers (loads self-pace ahead of scans)
NT = R // P        # row tiles per core = 8

_CACHE: dict = {}
LAST_RESULT = None  # BassKernelResults of the most recent run (for test.py)


def _build(w: float):
    from contextlib import ExitStack

    import concourse.bass as bass
    from concourse import mybir

    a = 1.0 - w
    f32 = mybir.dt.float32

    nc = bass.Bass()
    x_in = nc.dram_tensor("x", [R, T], f32, kind="ExternalInput")
    # init, host-pretransposed: s_in[p, i] = initial_state row 128*i + p
    s_in = nc.dram_tensor("init", [P, NT], f32, kind="ExternalInput")
    y_out = nc.dram_tensor("y", [R, T], f32, kind="ExternalOutput")

    with ExitStack() as ctx:
        ec = ctx.enter_context
        c_a = ec(nc.sbuf_tensor([P, T], f32))        # (1-w) broadcast tile
        sall = ec(nc.sbuf_tensor([P, NT], f32))      # init/w, DMA landing
        sall2 = ec(nc.sbuf_tensor([P, NT], f32))     # init/w, DVE-homed copy
        xts = [ec(nc.sbuf_tensor(f"xt{k}", [P, T], f32)) for k in range(XBUF)]
        zts = [ec(nc.sbuf_tensor(f"zt{k}", [P, T], f32)) for k in range(2)]
        yts = [ec(nc.sbuf_tensor(f"yt{k}", [P, T], f32)) for k in range(3)]
        # One sem per DMA buffer slot: at most one in-flight incrementer per
        # sem, so completion-order nondeterminism across concurrent DMAs
        # can't satisfy a wait with the wrong transfer.
        cst_sem = ec(nc.semaphore())  # c_a memset done
        cp_sem = ec(nc.semaphore())   # sall2 copy done
        s_sem = ec(nc.semaphore())    # init load done
        in_sems = [ec(nc.semaphore(f"in_sem{k}")) for k in range(XBUF)]
        scan_sem = ec(nc.semaphore()) # scan i done (z tile ready)
        act_sem = ec(nc.semaphore())  # ACT scale i done (y tile ready)
        out_sems = [ec(nc.semaphore(f"out_sem{k}")) for k in range(3)]
        block = ec(nc.Block())

        @block.sync
        def _(sync):
            # x loads on the SP HWDGE ring
            sync.dma_start(sall[:], s_in[:]).then_inc(s_sem, 16)
            for j in range(NT):
                if j >= XBUF:
                    sync.wait_ge(scan_sem, j - XBUF + 1)  # xt slot free
                sync.dma_start(
                    xts[j % XBUF][:], x_in[bass.ts(j, P), :]
                ).then_inc(in_sems[j % XBUF], 16)

        @block.vector
        def _(vector):
            # Engine pipelines are deep: even same-engine RAW/WAR hazards
            # need sem edges (the race detector enforces this).
            vector.memset(c_a[:], a).then_inc(cst_sem, 1)
            vector.wait_ge(s_sem, 16)
            vector.tensor_copy(sall2[:], sall[:]).then_inc(cp_sem, 1)
            vector.wait_ge(cst_sem, 1)
            vector.wait_ge(cp_sem, 1)
            for i in range(NT):
                vector.wait_ge(in_sems[i % XBUF], 16 * (i // XBUF + 1))
                if i >= 2:
                    vector.wait_ge(act_sem, i - 1)  # zt slot i%2 free
                # z[:, t] = a * z[:, t-1] + x[:, t], seeded with init/w
                vector.tensor_tensor_scan(
                    zts[i % 2][:], c_a[:], xts[i % XBUF][:], sall2[:, i : i + 1],
                    op0=mybir.AluOpType.mult, op1=mybir.AluOpType.add,
                ).then_inc(scan_sem, 1)

        @block.scalar
        def _(scalar):
            # y = w*z on ScalarE, then store on the ACT HWDGE ring
            for i in range(NT):
                scalar.wait_ge(scan_sem, i + 1)
                if i >= 3:
                    scalar.wait_ge(out_sems[i % 3], 16 * (i // 3))  # yt free
                scalar.mul(yts[i % 3][:], zts[i % 2][:], w).then_inc(act_sem, 1)
                scalar.wait_ge(act_sem, i + 1)
                scalar.dma_start(
                    y_out[bass.ts(i, P), :], yts[i % 3][:]
                ).then_inc(out_sems[i % 3], 16)
    return nc


def _run(in_maps, w: float, trace: bool = False):
    global LAST_RESULT
    from concourse.bass_utils import run_bass_kernel_spmd

    if w not in _CACHE:
        _CACHE[w] = _build(w)
    LAST_RESULT = run_bass_kernel_spmd(
        _CACHE[w], in_maps, list(range(M)), trace=trace
    )
    return LAST_RESULT.results


def kernel(mag_spec, initial_state, weights, _trace: bool = False) -> np.ndarray:
    w = float(np.clip(np.asarray(weights, dtype=np.float32).reshape(-1)[0], 0.0, 1.0))
    x = np.ascontiguousarray(np.asarray(mag_spec, dtype=np.float32)).reshape(B * C, T)
    s = np.asarray(initial_state, dtype=np.float32).reshape(B * C)
    if w == 0.0:
        # y_t = y_{t-1} = init for all t; the z = y/w formulation divides by w
        return np.broadcast_to(
            s.reshape(B, C, 1), (B, C, T)
        ).astype(np.float32).copy()
    # device scans z_t = x_t + (1-w) z_{t-1} seeded with init/w; y = w*z
    sw = (s / np.float32(w)).astype(np.float32)
    in_maps = [
        {
            "x": np.ascontiguousarray(x[i * R : (i + 1) * R]),
            "init": np.ascontiguousarray(sw[i * R : (i + 1) * R].reshape(NT, P).T),
        }
        for i in range(M)
    ]
    res = _run(in_maps, w, trace=_trace)
    y = np.concatenate([res[i]["y"] for i in range(M)], axis=0)
    return y.reshape(B, C, T)
